# revision 18
# baseline (speedup 1.0000x reference)
"""Trainium2 Bass kernel for nn_AttentionLayer_68547678044407.

Per-head sigmoid-QK exp-normalized attention with length masking.

Sharding: one head per NeuronCore (8 heads / 8 cores). Every core runs an
identical program over all batches (only the weight data differs per core),
so the SPMD contract is satisfied and the load is perfectly balanced.

Sequence sparsity: each batch is padded to a multiple of 128 rows on the
key/value side (t); the query side (s) is trimmed to the true length, so
work scales with sum(L_b * Lp_b) instead of B*S^2.

The exp over the score matrix is the ACT-engine wall (~93us at 1 elem/
cycle/lane), so it is split between two engines:
  - ScalarE (ACT): true exp via the LUT, scale = ln2/128.
  - VectorE (DVE): one-instruction Schraudolph exp2 -- scores arrive
    pre-scaled by 128*log2(e)/8 (folded into Q), so
    int16(round(z + B)) reinterpreted as bf16 approximates 2^t with
    ~2% rms error that largely cancels in the softmax ratio.

Math per (head h, batch b), with Lb = seq_lens[b]:
  Q^T,K^T = sigmoid(W^T x^T + b)       [64, Lp] each (bf16, stacked)
  Q is additionally scaled by 16*log2(e) for the exp trick
  V'      = x W_v + b_v (K=1 ones matmul), ones col  [Lp, 65]
  S^T     = exp2-ish(Q'^T.T K^T pairs) [128t, ns]  (ACT or DVE per tile)
  U'      = S~ @ V'                    [s, 65]     (col 64 = rowsum)
  O       = U'[:, :64] / (U'[:, 64] + 1e-8)
"""

import numpy as np

LAST_RESULT = None

import concourse.bacc as bacc
import concourse.bass as bass
import concourse.tile as tile
from concourse import mybir
from concourse.bass_utils import run_bass_kernel_spmd

H, D_IN, D_OUT = 8, 256, 64
B, S = 8, 2048
P = 128
NCORES = 8

BF16 = mybir.dt.bfloat16
FP32 = mybir.dt.float32
INT16 = mybir.dt.int16
AF = mybir.ActivationFunctionType

_BF16_NP = mybir.dt.np(BF16)

# columns per t-chunk slot in V' / U' (65 used, padded for 8B alignment)
VC = 72
# t-chunks fused per exp tile (psum tile spans G banks)
G = 2

# --- exp trick constants ---
# Q is pre-scaled by SCQ so PSUM scores hold t*128 with t = log2(exp score)
_SCQ = 128.0 * np.log2(np.e) / 8.0          # 23.083...
_ACT_SCALE = float(np.log(2.0) / 128.0)     # ACT: exp(scale*z) = 2^t
_SCHRAUD_C = 0.0547                          # L2-optimal mantissa shift
_SCHRAUD_B = float(128.0 * (127.0 - _SCHRAUD_C))
# fraction of exp elements routed to the DVE (tunable)
DVE_FRAC = 0.334


def _schedule(seq_lens):
    """Derive the static schedule from seq_lens (host-side)."""
    lens = [int(v) for v in seq_lens]
    chunks = [(l + P - 1) // P for l in lens]  # 128-row chunks per batch
    lp = [c * P for c in chunks]
    offs = np.concatenate([[0], np.cumsum(lp)]).astype(int)  # global row offset
    tsum = int(offs[-1])
    # projection blocks per batch (full padded range): (global_start, size)
    blocks = []
    for b in range(B):
        bb = []
        s0 = 0
        while s0 < lp[b]:
            ns = min(512, lp[b] - s0)
            bb.append((int(offs[b]) + s0, ns))
            s0 += ns
        blocks.append(bb)
    # query blocks per batch, trimmed to the true length
    sblocks = []
    for b in range(B):
        bb = []
        s0 = 0
        while s0 < lens[b]:
            ns = min(512, lens[b] - s0)
            bb.append((int(offs[b]) + s0, ns))
            s0 += ns
        sblocks.append(bb)
    return lens, chunks, lp, offs, tsum, blocks, sblocks


def _dve_plan(lens, chunks, sblocks, offs):
    """Global group counter: the chunk-level ACT/DVE exp split patterns off
    this index (see scores_phase)."""
    plan = {}
    gidx = 0
    for b in range(B):
        ngrp = (chunks[b] + G - 1) // G
        for bi, (s0, ns) in enumerate(sblocks[b]):
            for g in range(ngrp):
                plan[(b, bi, g)] = gidx
                gidx += 1
    return plan


def _build(nc, seq_lens):
    lens, chunks, lp, offs, tsum, blocks, sblocks = _schedule(seq_lens)
    nchunks = sum(chunks)
    plan = _dve_plan(lens, chunks, sblocks, offs)

    x_t = nc.dram_tensor("xt", [2 * P, tsum], BF16, kind="ExternalInput").ap()
    wqk = nc.dram_tensor("wqk", [2, P, P], BF16, kind="ExternalInput").ap()
    wv = nc.dram_tensor("wv", [2, P, D_OUT], BF16, kind="ExternalInput").ap()
    bqk = nc.dram_tensor("bqk", [P, 1], FP32, kind="ExternalInput").ap()
    bv1 = nc.dram_tensor("bv1", [1, D_OUT], BF16, kind="ExternalInput").ap()
    fin = nc.dram_tensor("fin", [P, 1], FP32, kind="ExternalInput").ap()
    tmask = nc.dram_tensor("tmask", [P, B], FP32, kind="ExternalInput").ap()
    o_out = nc.dram_tensor("o", [tsum, D_OUT], FP32, kind="ExternalOutput").ap()

    with tile.TileContext(nc) as tc:
        with (
            tc.tile_pool(name="big", bufs=1) as big,
            tc.tile_pool(name="stile", bufs=3) as spool,
            # st tiles live from exp until the (pipelined) U-phase one
            # s-block later — up to ~2 blocks x 8 groups in flight
            tc.tile_pool(name="stq", bufs=18) as stq,
            tc.tile_pool(name="opool", bufs=8) as opool,
            tc.tile_pool(name="fpool", bufs=4) as fpool,
            tc.tile_pool(name="ps_s", bufs=2, space="PSUM") as ps_s,
            tc.tile_pool(name="ps_a", bufs=2, space="PSUM") as ps_a,
            tc.tile_pool(name="ps_u", bufs=2, space="PSUM") as ps_u,
        ):
            # ---- persistent SBUF tensors ----
            xt_sb = big.tile([P, 2, tsum], BF16, tag="xt")
            qk_sb = big.tile([P, tsum], BF16, tag="qk")   # [q(0:64)|k(64:128), t]
            qk2_sb = big.tile([P, tsum], BF16, tag="qk2")  # swapped halves
            v_sb = big.tile([P, nchunks, VC], BF16, tag="v")
            wqk_sb = big.tile([P, 2, P], BF16, tag="wqk")
            wv_sb = big.tile([P, 2, D_OUT], BF16, tag="wv")
            bqk_sb = big.tile([P, 1], FP32, tag="bqk")
            bv1_sb = big.tile([1, D_OUT], BF16, tag="bv1")
            fin_sb = big.tile([P, 1], FP32, tag="fin")
            tm_sb = big.tile([P, B], FP32, tag="tmask")

            # small tensors first so the first projection isn't stuck behind
            # the bulk x loads in the queue
            nc.sync.dma_start(out=wqk_sb[:], in_=wqk.rearrange("c p m -> p c m"))
            nc.sync.dma_start(out=wv_sb[:], in_=wv.rearrange("c p m -> p c m"))
            nc.sync.dma_start(out=bqk_sb[:], in_=bqk)
            nc.sync.dma_start(out=bv1_sb[:], in_=bv1)
            nc.sync.dma_start(out=fin_sb[:], in_=fin)
            nc.sync.dma_start(out=tm_sb[:], in_=tmask)
            for b in range(B):
                for dc in range(2):
                    nc.gpsimd.dma_start(
                        out=xt_sb[:, dc, offs[b]:offs[b] + lp[b]],
                        in_=x_t[dc * P:(dc + 1) * P, offs[b]:offs[b] + lp[b]],
                    )

            # ones column of V' (col 64 of every chunk slot)
            nc.vector.memset(v_sb[:, :, 64:65], 1.0)
            # ones row (bias accumulate matmuls) + zero source (psum clears)
            one_sb = big.tile([1, P], BF16, tag="one")
            nc.vector.memset(one_sb[:], 1.0)
            zt_sb = big.tile([1, 512], BF16, tag="zt")
            nc.vector.memset(zt_sb[:], 0.0)
            wup_sb = big.tile([P, 512], BF16, tag="wup")
            nc.vector.memset(wup_sb[:], 0.0)

            # ~6.5us of dummy matmuls overlapping the x-load DMA ramp: gets
            # the PE HAM clock gate to 2.4 GHz before the real work arrives
            pwu = ps_s.tile([P, G, 512], FP32, tag="s")
            for i in range(30):
                nc.tensor.matmul(
                    pwu[:, i % 2, :],
                    lhsT=wup_sb[:, 0:P],
                    rhs=wup_sb[:],
                    start=True,
                    stop=True,
                )

            # ---- phase A: projections (emitted per batch, interleaved
            # with the pipelined phase B of earlier batches) ----
            def a_phase(b):
                for (s0, ns) in blocks[b]:
                    pqk = ps_a.tile([P, 512], FP32, tag="a")
                    for dc in range(2):
                        nc.tensor.matmul(
                            pqk[:, :ns],
                            lhsT=wqk_sb[:, dc, :],
                            rhs=xt_sb[:, dc, s0:s0 + ns],
                            start=(dc == 0),
                            stop=(dc == 1),
                        )
                    # sigmoid(x+b) = 0.5*tanh((x+b)/2) + 0.5 — tanh shares the
                    # exp table set, so no ACT table switching ever happens
                    th = spool.tile([P, 512], BF16, tag="th")
                    nc.scalar.activation(
                        out=th[:, :ns],
                        in_=pqk[:, :ns],
                        func=AF.Tanh,
                        bias=bqk_sb[:, 0:1],
                        scale=0.5,
                    )
                    # finish: q half gets 0.5*SCQ*th + 0.5*SCQ (exp2 folding),
                    # k half gets 0.5*th + 0.5 — per-partition scalar AP
                    nc.vector.tensor_scalar(
                        qk_sb[:, s0:s0 + ns], th[:, :ns],
                        fin_sb[:, 0:1], fin_sb[:, 0:1],
                        mybir.AluOpType.mult, mybir.AluOpType.add,
                    )
                    # swapped copy: K^T to partitions 0:64, Q^T to 64:128
                    nc.sync.dma_start(
                        out=qk2_sb[0:64, s0:s0 + ns],
                        in_=qk_sb[64:128, s0:s0 + ns],
                    )
                    nc.sync.dma_start(
                        out=qk2_sb[64:128, s0:s0 + ns],
                        in_=qk_sb[0:64, s0:s0 + ns],
                    )

                # V' projection: groups of up to 4 chunks share one psum bank
                c0 = offs[b] // P
                ci = 0
                while ci < chunks[b]:
                    cg = min(4, chunks[b] - ci)
                    pv = ps_a.tile([P, 4, D_OUT], FP32, tag="a")
                    nc.tensor.matmul(
                        pv.rearrange("p a b -> p (a b)")[:, 0:4 * D_OUT],
                        lhsT=zt_sb[0:1, 0:P],
                        rhs=zt_sb[0:1, 0:4 * D_OUT],
                        start=True,
                        stop=False,
                    )
                    for i in range(cg):
                        t0 = offs[b] + (ci + i) * P
                        for dc in range(2):
                            nc.tensor.matmul(
                                pv[:, i, :],
                                lhsT=xt_sb[:, dc, t0:t0 + P],
                                rhs=wv_sb[:, dc, :],
                                start=False,
                                stop=False,
                            )
                        # bias add: ones_col^T @ bv row accumulates b_v
                        # (stop only on the last matmul of the whole bank —
                        # stop clears the zero-region tracking for the bank)
                        nc.tensor.matmul(
                            pv[:, i, :],
                            lhsT=one_sb[0:1, 0:P],
                            rhs=bv1_sb[0:1, :],
                            start=False,
                            stop=(i == cg - 1),
                        )
                    nc.vector.tensor_copy(
                        v_sb[:, c0 + ci:c0 + ci + cg, 0:D_OUT],
                        pv[:, 0:cg, :],
                    )
                    ci += cg
                # zero pad rows of the last chunk (t in [len, lp))
                if lens[b] % P != 0:
                    cl = c0 + chunks[b] - 1
                    nc.vector.tensor_scalar_mul(
                        v_sb[:, cl, 0:65], v_sb[:, cl, 0:65], tm_sb[:, b:b + 1]
                    )

            # ---- phase B: attention, software-pipelined over s-blocks ----
            # For each s-block: emit all score matmuls + exp tiles first;
            # the U-phase (S~ @ V' accumulation + normalize + store) of the
            # PREVIOUS s-block is emitted after, so the PE always has a dense
            # backlog and the HAM clock gate stays at full speed.
            def scores_phase(b, bi, s0, ns):
                c0 = offs[b] // P
                ngrp = (chunks[b] + G - 1) // G
                pu = ps_u.tile([P, 4, VC], FP32, tag="u")
                nc.tensor.matmul(
                    pu.rearrange("p a b -> p (a b)")[0:min(ns, P), 0:4 * VC],
                    lhsT=zt_sb[0:1, 0:min(ns, P)],
                    rhs=zt_sb[0:1, 0:4 * VC],
                    start=True,
                    stop=False,
                )
                sts = []
                for g in range(ngrp):
                    cg = min(G, chunks[b] - g * G)
                    pst = ps_s.tile([P, G, 512], FP32, tag="s")
                    for k in range(cg):
                        ci = g * G + k
                        t0 = offs[b] + ci * P
                        half = ci % 2
                        lhsT = (qk2_sb if half == 0 else qk_sb)[
                            half * 64:half * 64 + 64, t0:t0 + P
                        ]
                        rhs = (qk_sb if half == 0 else qk2_sb)[
                            half * 64:half * 64 + 64, s0:s0 + ns
                        ]
                        nc.tensor.matmul(
                            pst[:, k, :ns],
                            lhsT=lhsT,
                            rhs=rhs,
                            start=True,
                            stop=True,
                            tile_position=(half * 64, 0),
                        )
                    st = stq.tile([P, G, 512], BF16, tag="st")
                    # exp split at chunk granularity: in 2 of every 3 groups
                    # the DVE takes one chunk (Schraudolph) while ACT takes
                    # the other concurrently — the pst->st handoff latency
                    # roughly halves (so score matmuls never stall on a psum
                    # slot) and each output row still sees a ~1/3 mix of
                    # DVE-approximated t-chunks, which lets the exp error
                    # cancel inside each row's softmax.
                    gi = plan[(b, bi, g)]
                    dve_k = -1
                    if cg == 2 and gi % 3 != 0:
                        dve_k = gi % 2
                    elif cg == 1 and gi % 3 == 1:
                        dve_k = 0
                    for k in range(cg):
                        if k == dve_k:
                            # DVE Schraudolph: int16(z + B) bits == bf16 2^t
                            nc.vector.tensor_scalar(
                                st[:, k, :ns].bitcast(INT16),
                                pst[:, k, :ns],
                                _SCHRAUD_B, None,
                                mybir.AluOpType.add,
                            )
                        elif k == (dve_k ^ 1) and cg == 2 and dve_k >= 0:
                            nc.scalar.activation(
                                out=st[:, k, :ns],
                                in_=pst[:, k, :ns],
                                func=AF.Exp,
                                scale=_ACT_SCALE,
                            )
                    if dve_k < 0:
                        nc.scalar.activation(
                            out=st[:, 0:cg, :ns],
                            in_=pst[:, 0:cg, :ns],
                            func=AF.Exp,
                            scale=_ACT_SCALE,
                        )
                    sts.append((g, cg, st))
                return (b, s0, ns, pu, sts)

            def u_phase(stage):
                b, s0, ns, pu, sts = stage
                c0 = offs[b] // P
                nsub = (ns + P - 1) // P
                for g, cg, st in sts:
                    for k in range(cg):
                        ci = g * G + k
                        for j in range(nsub):
                            m = min(P, ns - j * P)
                            nc.tensor.matmul(
                                pu[0:m, j, 0:65],
                                lhsT=st[:, k, j * P:j * P + m],
                                rhs=v_sb[:, c0 + ci, 0:65],
                                start=False,
                                stop=False,
                            )
                # dummy zero-add matmul closes the accumulation group over
                # the full bank span (partial-m U matmuls can't — the
                # region flags are partition-scoped); WAW deps force it after
                # every U matmul, and the normalization reads after it
                nc.tensor.matmul(
                    pu.rearrange("p a b -> p (a b)")[0:min(ns, P), 0:4 * VC],
                    lhsT=zt_sb[0:1, 0:min(ns, P)],
                    rhs=zt_sb[0:1, 0:4 * VC],
                    start=False,
                    stop=True,
                )
                # normalization: r = 1/(rowsum + 1e-8), out = U * r
                pl = min(ns, P)
                rt = fpool.tile([P, 4], FP32, tag="r")
                nc.vector.tensor_scalar_add(
                    rt[0:pl, 0:nsub],
                    pu[0:pl, 0:nsub, 64:65].squeeze(2), 1e-8)
                nc.vector.reciprocal(rt[0:pl, 0:nsub], rt[0:pl, 0:nsub])
                ob = opool.tile([P, 4, D_OUT], FP32, tag="o")
                for j in range(nsub):
                    nc.vector.tensor_scalar_mul(
                        ob[0:pl, j, :], pu[0:pl, j, 0:D_OUT],
                        rt[0:pl, j:j + 1])
                nfull = ns // P
                if nfull:
                    nc.gpsimd.dma_start(
                        out=o_out[s0:s0 + nfull * P, :].rearrange(
                            "(j p) e -> p j e", p=P),
                        in_=ob[:, 0:nfull, :],
                    )
                mt = ns - nfull * P
                if mt:
                    nc.gpsimd.dma_start(
                        out=o_out[s0 + nfull * P:s0 + ns, :],
                        in_=ob[0:mt, nfull, :],
                    )

            pending = None
            for b in range(B):
                a_phase(b)
                for bi, (s0, ns) in enumerate(sblocks[b]):
                    stage = scores_phase(b, bi, s0, ns)
                    if pending is not None:
                        u_phase(pending)
                    pending = stage
            if pending is not None:
                u_phase(pending)
    return nc


def kernel(**inputs):
    x = np.asarray(inputs["x_text"], dtype=np.float32)
    seq_lens = np.asarray(inputs["seq_lens"]).astype(np.int64)
    wq = np.asarray(inputs["Wq"], dtype=np.float32)
    bq = np.asarray(inputs["bq"], dtype=np.float32)
    wk = np.asarray(inputs["Wk"], dtype=np.float32)
    bk = np.asarray(inputs["bk"], dtype=np.float32)
    wv = np.asarray(inputs["Wv"], dtype=np.float32)
    bv = np.asarray(inputs["bv"], dtype=np.float32)

    lens, chunks, lp, offs, tsum, blocks, sblocks = _schedule(seq_lens)

    nc = bacc.Bacc("TRN2", target_bir_lowering=False, debug=False,
                   num_devices=NCORES)
    _build(nc, seq_lens)
    nc.finalize()

    # host-side packing: x^T per batch, padded to lp[b], concatenated
    xt = np.zeros((2 * P, tsum), dtype=_BF16_NP)
    for b in range(B):
        l = lens[b]
        xt[:, offs[b]:offs[b] + l] = x[b, :l, :].T.astype(_BF16_NP)

    # per-batch tail mask: partition p valid iff p < len % 128 (for last chunk)
    tmask = np.zeros((P, B), dtype=np.float32)
    for b in range(B):
        rem = lens[b] % P
        tmask[:rem if rem else P, b] = 1.0

    # finishing scalars: q half scaled for the exp2 trick, k half plain
    fin = np.empty((P, 1), dtype=np.float32)
    fin[0:64] = 0.5 * _SCQ
    fin[64:128] = 0.5

    in_maps = []
    for h in range(H):
        wqk = np.concatenate([wq[h], wk[h]], axis=1)  # [256, 128]
        in_maps.append({
            "xt": xt,
            "wqk": np.ascontiguousarray(
                wqk.reshape(2, P, P).astype(_BF16_NP)),
            "wv": np.ascontiguousarray(
                wv[h].reshape(2, P, D_OUT).astype(_BF16_NP)),
            # tanh-form sigmoid needs bias/2
            "bqk": (np.concatenate([bq[h], bk[h]]).reshape(P, 1) * 0.5)
                     .astype(np.float32),
            "bv1": bv[h].reshape(1, D_OUT).astype(_BF16_NP),
            "fin": fin,
            "tmask": tmask,
        })

    res = run_bass_kernel_spmd(nc, in_maps, list(range(NCORES)))
    global LAST_RESULT
    LAST_RESULT = res

    out = np.zeros((B, S, H * D_OUT), dtype=np.float32)
    for h in range(H):
        o = res.results[h]["o"]
        for b in range(B):
            l = lens[b]
            out[b, :l, h * D_OUT:(h + 1) * D_OUT] = o[offs[b]:offs[b] + l]
    return out


# revision 20
# speedup vs baseline: 1.4599x; 1.4599x over previous
"""Trainium2 Bass kernel for nn_AttentionLayer_68547678044407.

Per-head sigmoid-QK exp-normalized attention with length masking.

Sharding: one head per NeuronCore (8 heads / 8 cores). Every core runs an
identical program over all batches (only the weight data differs per core),
so the SPMD contract is satisfied and the load is perfectly balanced.

Sequence sparsity: each batch is padded to a multiple of 128 rows on the
key/value side (t); the query side (s) is trimmed to the true length, so
work scales with sum(L_b * Lp_b) instead of B*S^2.

The exp over the score matrix is the ACT-engine wall (~93us at 1 elem/
cycle/lane), so it is split between two engines:
  - ScalarE (ACT): true exp via the LUT, scale = ln2/128.
  - VectorE (DVE): one-instruction Schraudolph exp2 -- scores arrive
    pre-scaled by 128*log2(e)/8 (folded into Q), so
    int16(round(z + B)) reinterpreted as bf16 approximates 2^t with
    ~2% rms error that largely cancels in the softmax ratio.

Math per (head h, batch b), with Lb = seq_lens[b]:
  Q^T,K^T = sigmoid(W^T x^T + b)       [64, Lp] each (bf16, stacked)
  Q is additionally scaled by 16*log2(e) for the exp trick
  V'      = x W_v + b_v (K=1 ones matmul), ones col  [Lp, 65]
  S^T     = exp2-ish(Q'^T.T K^T pairs) [128t, ns]  (ACT or DVE per tile)
  U'      = S~ @ V'                    [s, 65]     (col 64 = rowsum)
  O       = U'[:, :64] / (U'[:, 64] + 1e-8)
"""

import numpy as np

LAST_RESULT = None

import concourse.bacc as bacc
import concourse.bass as bass
import concourse.tile as tile
from concourse import mybir
from concourse.bass_utils import run_bass_kernel_spmd

H, D_IN, D_OUT = 8, 256, 64
B, S = 8, 2048
P = 128
NCORES = 8

BF16 = mybir.dt.bfloat16
FP32 = mybir.dt.float32
INT16 = mybir.dt.int16
AF = mybir.ActivationFunctionType

_BF16_NP = mybir.dt.np(BF16)

# columns per t-chunk slot in V' / U' (65 used, padded for 8B alignment)
VC = 72
# t-chunks fused per exp tile (psum tile spans G banks)
G = 2

# --- exp trick constants ---
# Q is pre-scaled by SCQ so PSUM scores hold t*128 with t = log2(exp score)
_SCQ = 128.0 * np.log2(np.e) / 8.0          # 23.083...
_ACT_SCALE = float(np.log(2.0) / 128.0)     # ACT: exp(scale*z) = 2^t
_SCHRAUD_C = 0.0547                          # L2-optimal mantissa shift
_SCHRAUD_B = float(128.0 * (127.0 - _SCHRAUD_C))
# fraction of exp elements routed to the DVE (tunable)
DVE_FRAC = 0.334


def _schedule(seq_lens):
    """Derive the static schedule from seq_lens (host-side)."""
    lens = [int(v) for v in seq_lens]
    chunks = [(l + P - 1) // P for l in lens]  # 128-row chunks per batch
    lp = [c * P for c in chunks]
    offs = np.concatenate([[0], np.cumsum(lp)]).astype(int)  # global row offset
    tsum = int(offs[-1])
    # projection blocks per batch (full padded range): (global_start, size)
    blocks = []
    for b in range(B):
        bb = []
        s0 = 0
        while s0 < lp[b]:
            ns = min(512, lp[b] - s0)
            bb.append((int(offs[b]) + s0, ns))
            s0 += ns
        blocks.append(bb)
    # query blocks per batch, trimmed to the true length
    sblocks = []
    for b in range(B):
        bb = []
        s0 = 0
        while s0 < lens[b]:
            ns = min(512, lens[b] - s0)
            bb.append((int(offs[b]) + s0, ns))
            s0 += ns
        sblocks.append(bb)
    return lens, chunks, lp, offs, tsum, blocks, sblocks


def _dve_plan(lens, chunks, sblocks, offs):
    """Global group counter: the chunk-level ACT/DVE exp split patterns off
    this index (see scores_phase)."""
    plan = {}
    gidx = 0
    for b in range(B):
        ngrp = (chunks[b] + G - 1) // G
        for bi, (s0, ns) in enumerate(sblocks[b]):
            for g in range(ngrp):
                plan[(b, bi, g)] = gidx
                gidx += 1
    return plan


def _build(nc, seq_lens):
    lens, chunks, lp, offs, tsum, blocks, sblocks = _schedule(seq_lens)
    nchunks = sum(chunks)
    plan = _dve_plan(lens, chunks, sblocks, offs)

    x_t = nc.dram_tensor("xt", [2 * P, tsum], BF16, kind="ExternalInput").ap()
    wqk = nc.dram_tensor("wqk", [2, P, P], BF16, kind="ExternalInput").ap()
    wv = nc.dram_tensor("wv", [2, P, D_OUT], BF16, kind="ExternalInput").ap()
    bqk = nc.dram_tensor("bqk", [P, 1], FP32, kind="ExternalInput").ap()
    bv1 = nc.dram_tensor("bv1", [1, D_OUT], BF16, kind="ExternalInput").ap()
    fin = nc.dram_tensor("fin", [P, 1], FP32, kind="ExternalInput").ap()
    tmask = nc.dram_tensor("tmask", [P, B], FP32, kind="ExternalInput").ap()
    o_out = nc.dram_tensor("o", [tsum, D_OUT], FP32, kind="ExternalOutput").ap()

    with tile.TileContext(nc) as tc:
        with (
            tc.tile_pool(name="big", bufs=1) as big,
            tc.tile_pool(name="stile", bufs=3) as spool,
            # st tiles live from exp until the (pipelined) U-phase one
            # s-block later — up to ~2 blocks x 8 groups in flight
            tc.tile_pool(name="stq", bufs=18) as stq,
            tc.tile_pool(name="opool", bufs=8) as opool,
            tc.tile_pool(name="fpool", bufs=4) as fpool,
            tc.tile_pool(name="ps_s", bufs=2, space="PSUM") as ps_s,
            tc.tile_pool(name="ps_a", bufs=2, space="PSUM") as ps_a,
            tc.tile_pool(name="ps_u", bufs=2, space="PSUM") as ps_u,
        ):
            # ---- persistent SBUF tensors ----
            xt_sb = big.tile([P, 2, tsum], BF16, tag="xt")
            qk_sb = big.tile([P, tsum], BF16, tag="qk")   # [q(0:64)|k(64:128), t]
            qk2_sb = big.tile([P, tsum], BF16, tag="qk2")  # swapped halves
            v_sb = big.tile([P, nchunks, VC], BF16, tag="v")
            wqk_sb = big.tile([P, 2, P], BF16, tag="wqk")
            wv_sb = big.tile([P, 2, D_OUT], BF16, tag="wv")
            bqk_sb = big.tile([P, 1], FP32, tag="bqk")
            bv1_sb = big.tile([1, D_OUT], BF16, tag="bv1")
            fin_sb = big.tile([P, 1], FP32, tag="fin")
            tm_sb = big.tile([P, B], FP32, tag="tmask")

            # small tensors first so the first projection isn't stuck behind
            # the bulk x loads in the queue
            nc.sync.dma_start(out=wqk_sb[:], in_=wqk.rearrange("c p m -> p c m"))
            nc.sync.dma_start(out=wv_sb[:], in_=wv.rearrange("c p m -> p c m"))
            nc.sync.dma_start(out=bqk_sb[:], in_=bqk)
            nc.sync.dma_start(out=bv1_sb[:], in_=bv1)
            nc.sync.dma_start(out=fin_sb[:], in_=fin)
            nc.sync.dma_start(out=tm_sb[:], in_=tmask)
            for b in range(B):
                for dc in range(2):
                    nc.gpsimd.dma_start(
                        out=xt_sb[:, dc, offs[b]:offs[b] + lp[b]],
                        in_=x_t[dc * P:(dc + 1) * P, offs[b]:offs[b] + lp[b]],
                    )

            # ones column of V' (col 64 of every chunk slot)
            nc.vector.memset(v_sb[:, :, 64:65], 1.0)
            # ones row (bias accumulate matmuls) + zero source (psum clears)
            one_sb = big.tile([1, P], BF16, tag="one")
            nc.vector.memset(one_sb[:], 1.0)
            zt_sb = big.tile([1, 512], BF16, tag="zt")
            nc.vector.memset(zt_sb[:], 0.0)
            wup_sb = big.tile([P, 512], BF16, tag="wup")
            nc.vector.memset(wup_sb[:], 0.0)

            # ~6.5us of dummy matmuls overlapping the x-load DMA ramp: gets
            # the PE HAM clock gate to 2.4 GHz before the real work arrives
            pwu = ps_s.tile([P, G, 512], FP32, tag="s")
            for i in range(30):
                nc.tensor.matmul(
                    pwu[:, i % 2, :],
                    lhsT=wup_sb[:, 0:P],
                    rhs=wup_sb[:],
                    start=True,
                    stop=True,
                )

            # ---- phase A: projections (emitted per batch, interleaved
            # with the pipelined phase B of earlier batches) ----
            def a_phase(b):
                for (s0, ns) in blocks[b]:
                    pqk = ps_a.tile([P, 512], FP32, tag="a")
                    for dc in range(2):
                        nc.tensor.matmul(
                            pqk[:, :ns],
                            lhsT=wqk_sb[:, dc, :],
                            rhs=xt_sb[:, dc, s0:s0 + ns],
                            start=(dc == 0),
                            stop=(dc == 1),
                        )
                    # sigmoid(x+b) = 0.5*tanh((x+b)/2) + 0.5 — tanh shares the
                    # exp table set, so no ACT table switching ever happens
                    th = spool.tile([P, 512], BF16, tag="th")
                    nc.scalar.activation(
                        out=th[:, :ns],
                        in_=pqk[:, :ns],
                        func=AF.Tanh,
                        bias=bqk_sb[:, 0:1],
                        scale=0.5,
                    )
                    # finish: q half gets 0.5*SCQ*th + 0.5*SCQ (exp2 folding),
                    # k half gets 0.5*th + 0.5 — per-partition scalar AP
                    nc.vector.tensor_scalar(
                        qk_sb[:, s0:s0 + ns], th[:, :ns],
                        fin_sb[:, 0:1], fin_sb[:, 0:1],
                        mybir.AluOpType.mult, mybir.AluOpType.add,
                    )
                    # swapped copy: K^T to partitions 0:64, Q^T to 64:128
                    nc.sync.dma_start(
                        out=qk2_sb[0:64, s0:s0 + ns],
                        in_=qk_sb[64:128, s0:s0 + ns],
                    )
                    nc.sync.dma_start(
                        out=qk2_sb[64:128, s0:s0 + ns],
                        in_=qk_sb[0:64, s0:s0 + ns],
                    )

                # V' projection: groups of up to 4 chunks share one psum bank
                c0 = offs[b] // P
                ci = 0
                while ci < chunks[b]:
                    cg = min(4, chunks[b] - ci)
                    pv = ps_a.tile([P, 4, D_OUT], FP32, tag="a")
                    # start=True on the first matmul clears has_written for
                    # the whole bank; later chunks overwrite-on-first-touch
                    for i in range(cg):
                        t0 = offs[b] + (ci + i) * P
                        for dc in range(2):
                            nc.tensor.matmul(
                                pv[:, i, :],
                                lhsT=xt_sb[:, dc, t0:t0 + P],
                                rhs=wv_sb[:, dc, :],
                                start=(i == 0 and dc == 0),
                                stop=False,
                            )
                        # bias add: ones_col^T @ bv row accumulates b_v
                        # (stop only on the last matmul of the whole bank —
                        # stop clears the zero-region tracking for the bank)
                        nc.tensor.matmul(
                            pv[:, i, :],
                            lhsT=one_sb[0:1, 0:P],
                            rhs=bv1_sb[0:1, :],
                            start=False,
                            stop=(i == cg - 1),
                        )
                    nc.vector.tensor_copy(
                        v_sb[:, c0 + ci:c0 + ci + cg, 0:D_OUT],
                        pv[:, 0:cg, :],
                    )
                    ci += cg
                # zero pad rows of the last chunk (t in [len, lp))
                if lens[b] % P != 0:
                    cl = c0 + chunks[b] - 1
                    nc.vector.tensor_scalar_mul(
                        v_sb[:, cl, 0:65], v_sb[:, cl, 0:65], tm_sb[:, b:b + 1]
                    )

            # ---- phase B: attention, software-pipelined over s-blocks ----
            # For each s-block: emit all score matmuls + exp tiles first;
            # the U-phase (S~ @ V' accumulation + normalize + store) of the
            # PREVIOUS s-block is emitted after, so the PE always has a dense
            # backlog and the HAM clock gate stays at full speed.
            def scores_phase(b, bi, s0, ns):
                c0 = offs[b] // P
                ngrp = (chunks[b] + G - 1) // G
                pu = ps_u.tile([P, 4, VC], FP32, tag="u")
                sts = []
                for g in range(ngrp):
                    cg = min(G, chunks[b] - g * G)
                    pst = ps_s.tile([P, G, 512], FP32, tag="s")
                    for k in range(cg):
                        ci = g * G + k
                        t0 = offs[b] + ci * P
                        half = ci % 2
                        lhsT = (qk2_sb if half == 0 else qk_sb)[
                            half * 64:half * 64 + 64, t0:t0 + P
                        ]
                        rhs = (qk_sb if half == 0 else qk2_sb)[
                            half * 64:half * 64 + 64, s0:s0 + ns
                        ]
                        nc.tensor.matmul(
                            pst[:, k, :ns],
                            lhsT=lhsT,
                            rhs=rhs,
                            start=True,
                            stop=True,
                            tile_position=(half * 64, 0),
                        )
                    st = stq.tile([P, G, 512], BF16, tag="st")
                    # exp split at chunk granularity: in 2 of every 3 groups
                    # the DVE takes one chunk (Schraudolph) while ACT takes
                    # the other concurrently — the pst->st handoff latency
                    # roughly halves (so score matmuls never stall on a psum
                    # slot) and each output row still sees a ~1/3 mix of
                    # DVE-approximated t-chunks, which lets the exp error
                    # cancel inside each row's softmax.
                    gi = plan[(b, bi, g)]
                    dve_k = -1
                    if cg == 2 and gi % 3 != 0:
                        dve_k = gi % 2
                    elif cg == 1 and gi % 3 == 1:
                        dve_k = 0
                    for k in range(cg):
                        if k == dve_k:
                            # DVE Schraudolph: int16(z + B) bits == bf16 2^t
                            nc.vector.tensor_scalar(
                                st[:, k, :ns].bitcast(INT16),
                                pst[:, k, :ns],
                                _SCHRAUD_B, None,
                                mybir.AluOpType.add,
                            )
                        elif k == (dve_k ^ 1) and cg == 2 and dve_k >= 0:
                            nc.scalar.activation(
                                out=st[:, k, :ns],
                                in_=pst[:, k, :ns],
                                func=AF.Exp,
                                scale=_ACT_SCALE,
                            )
                    if dve_k < 0:
                        nc.scalar.activation(
                            out=st[:, 0:cg, :ns],
                            in_=pst[:, 0:cg, :ns],
                            func=AF.Exp,
                            scale=_ACT_SCALE,
                        )
                    sts.append((g, cg, st))
                return (b, s0, ns, pu, sts)

            def u_phase(stage):
                b, s0, ns, pu, sts = stage
                c0 = offs[b] // P
                nsub = (ns + P - 1) // P
                first = True
                for gi_, (g, cg, st) in enumerate(sts):
                    last_grp = gi_ == len(sts) - 1
                    for k in range(cg):
                        ci = g * G + k
                        last_chunk = last_grp and k == cg - 1
                        # the final matmul must span the same partitions as
                        # the start matmul (region flags are partition-
                        # scoped), so the last chunk runs j in reverse: its
                        # j=0 (full-span) matmul carries stop=True
                        js = range(nsub - 1, -1, -1) if last_chunk                             else range(nsub)
                        for j in js:
                            m = min(P, ns - j * P)
                            nc.tensor.matmul(
                                pu[0:m, j, 0:65],
                                lhsT=st[:, k, j * P:j * P + m],
                                rhs=v_sb[:, c0 + ci, 0:65],
                                start=first,
                                stop=(last_chunk and j == 0),
                            )
                            first = False
                # normalization: r = 1/(rowsum + 1e-8), out = U * r.
                # Only partitions the U matmuls actually wrote are touched
                # (a trailing partial j-chunk covers m < 128 partitions).
                mlast = ns - (nsub - 1) * P
                nf = nsub if mlast == P else nsub - 1
                rt = fpool.tile([P, 4], FP32, tag="r")
                if nf:
                    nc.vector.tensor_scalar_add(
                        rt[:, 0:nf], pu[:, 0:nf, 64:65].squeeze(2), 1e-8)
                    nc.vector.reciprocal(rt[:, 0:nf], rt[:, 0:nf])
                if nf < nsub:
                    nc.vector.tensor_scalar_add(
                        rt[0:mlast, nf:nsub],
                        pu[0:mlast, nf:nsub, 64:65].squeeze(2), 1e-8)
                    nc.vector.reciprocal(rt[0:mlast, nf:nsub],
                                         rt[0:mlast, nf:nsub])
                ob = opool.tile([P, 4, D_OUT], FP32, tag="o")
                for j in range(nsub):
                    m = min(P, ns - j * P)
                    nc.vector.tensor_scalar_mul(
                        ob[0:m, j, :], pu[0:m, j, 0:D_OUT],
                        rt[0:m, j:j + 1])
                nfull = ns // P
                if nfull:
                    nc.gpsimd.dma_start(
                        out=o_out[s0:s0 + nfull * P, :].rearrange(
                            "(j p) e -> p j e", p=P),
                        in_=ob[:, 0:nfull, :],
                    )
                mt = ns - nfull * P
                if mt:
                    nc.gpsimd.dma_start(
                        out=o_out[s0 + nfull * P:s0 + ns, :],
                        in_=ob[0:mt, nfull, :],
                    )

            pending = None
            for b in range(B):
                a_phase(b)
                for bi, (s0, ns) in enumerate(sblocks[b]):
                    stage = scores_phase(b, bi, s0, ns)
                    if pending is not None:
                        u_phase(pending)
                    pending = stage
            if pending is not None:
                u_phase(pending)
    return nc


def kernel(**inputs):
    x = np.asarray(inputs["x_text"], dtype=np.float32)
    seq_lens = np.asarray(inputs["seq_lens"]).astype(np.int64)
    wq = np.asarray(inputs["Wq"], dtype=np.float32)
    bq = np.asarray(inputs["bq"], dtype=np.float32)
    wk = np.asarray(inputs["Wk"], dtype=np.float32)
    bk = np.asarray(inputs["bk"], dtype=np.float32)
    wv = np.asarray(inputs["Wv"], dtype=np.float32)
    bv = np.asarray(inputs["bv"], dtype=np.float32)

    lens, chunks, lp, offs, tsum, blocks, sblocks = _schedule(seq_lens)

    nc = bacc.Bacc("TRN2", target_bir_lowering=False, debug=False,
                   num_devices=NCORES)
    _build(nc, seq_lens)
    nc.finalize()

    # host-side packing: x^T per batch, padded to lp[b], concatenated
    xt = np.zeros((2 * P, tsum), dtype=_BF16_NP)
    for b in range(B):
        l = lens[b]
        xt[:, offs[b]:offs[b] + l] = x[b, :l, :].T.astype(_BF16_NP)

    # per-batch tail mask: partition p valid iff p < len % 128 (for last chunk)
    tmask = np.zeros((P, B), dtype=np.float32)
    for b in range(B):
        rem = lens[b] % P
        tmask[:rem if rem else P, b] = 1.0

    # finishing scalars: q half scaled for the exp2 trick, k half plain
    fin = np.empty((P, 1), dtype=np.float32)
    fin[0:64] = 0.5 * _SCQ
    fin[64:128] = 0.5

    in_maps = []
    for h in range(H):
        wqk = np.concatenate([wq[h], wk[h]], axis=1)  # [256, 128]
        in_maps.append({
            "xt": xt,
            "wqk": np.ascontiguousarray(
                wqk.reshape(2, P, P).astype(_BF16_NP)),
            "wv": np.ascontiguousarray(
                wv[h].reshape(2, P, D_OUT).astype(_BF16_NP)),
            # tanh-form sigmoid needs bias/2
            "bqk": (np.concatenate([bq[h], bk[h]]).reshape(P, 1) * 0.5)
                     .astype(np.float32),
            "bv1": bv[h].reshape(1, D_OUT).astype(_BF16_NP),
            "fin": fin,
            "tmask": tmask,
        })

    res = run_bass_kernel_spmd(nc, in_maps, list(range(NCORES)))
    global LAST_RESULT
    LAST_RESULT = res

    out = np.zeros((B, S, H * D_OUT), dtype=np.float32)
    for h in range(H):
        o = res.results[h]["o"]
        for b in range(B):
            l = lens[b]
            out[b, :l, h * D_OUT:(h + 1) * D_OUT] = o[offs[b]:offs[b] + l]
    return out
